# revision 34
# baseline (speedup 1.0000x reference)
"""Trainium2 Bass kernel for nn_Attention_33595234189924.

Multi-head attention (B=2, S=2048, D=2048, H=16, hd=128) with RoPE,
tensor-parallel over heads: 8 cores x 2 heads each.

v2 schedule: single software-pipelined stream.
  - per token chunk: qk and v PSUM chains interleaved (1 PSUM buf each),
    drains on ACT / DVE; RoPE in bf16 on DVE (2x mode).
  - attention quanta (scores->exp->PV per 2 key tiles) interleaved into
    every projection window as soon as their chunk deps are met
    (no max-subtraction needed: softmax normalizes at close).
  - 4 in-flight combo accumulators in PSUM; rowsum via ones-matmul into
    a scores-pool slot; probs tree-summed on DVE.
  - drain phase after projections runs leftover quanta with
    double-buffered score banks and delayed closes.
"""

import os
import sys
from collections import deque

sys.path.insert(0, "/opt/trn_rl_repo")

import numpy as np
import ml_dtypes

import concourse.bass as bass
import concourse.tile as tile
from concourse import bacc, mybir
from concourse.bass import ts
from concourse.bass_utils import run_bass_kernel_spmd

# If anything enables tracing (e.g. BASS_TRACE in the environment) and the
# image's antenv lacks axon_hooks, run_bass_kernel_spmd would crash on
# import. Register a null hook so it degrades to the untraced path.
try:
    from antenv import axon_hooks as _ah  # noqa: F401
except Exception:
    import types as _types

    _m = _types.ModuleType("antenv.axon_hooks")
    _m.get_axon_ntff_profile_hook = lambda: None
    _m.set_axon_ntff_profile_hook = lambda hook: None
    sys.modules["antenv.axon_hooks"] = _m

B, S, D, H = 2, 2048, 2048, 16
HD = 128
T = B * S
NCORES = 8
NKT = D // 128        # contraction tiles for projections
CHUNK = 512           # token chunk in projection phase
QCHUNK = 512          # query chunk in attention phase
NJ = S // 128         # key tiles per batch
SCALE = 1.0 / float(np.sqrt(HD))

F32 = mybir.dt.float32
BF16 = mybir.dt.bfloat16
Exp = mybir.ActivationFunctionType.Exp

_prog_cache = {}
_last_results = {}


def _build_program():
    if "nc" in _prog_cache:
        return _prog_cache["nc"]

    nc = bacc.Bacc("TRN2", target_bir_lowering=False, debug=False,
                   num_devices=NCORES)

    xT = nc.dram_tensor("xT", [D, T], BF16, kind="ExternalInput").ap()
    wqkT = nc.dram_tensor("wqkT", [D, 512], BF16, kind="ExternalInput").ap()
    wvT = nc.dram_tensor("wvT", [D, 256], BF16, kind="ExternalInput").ap()
    bqk_d = nc.dram_tensor("bqk", [128, 4], F32, kind="ExternalInput").ap()
    bqksw_d = nc.dram_tensor("bqksw", [128, 4], F32, kind="ExternalInput").ap()
    bv_d = nc.dram_tensor("bv", [128, 2], F32, kind="ExternalInput").ap()
    cos_d = nc.dram_tensor("cosg", [128, S], BF16, kind="ExternalInput").ap()
    sin_d = nc.dram_tensor("sing", [128, S], BF16, kind="ExternalInput").ap()
    out_d = nc.dram_tensor("out", [256, T], F32, kind="ExternalOutput").ap()

    with tile.TileContext(nc) as tc:
        with tc.tile_pool(name="singles", bufs=1) as singles:
            # wqk streams kt-by-kt on the (otherwise idle) vector queue so the
            # first projection chain can start as soon as kt0 lands; all
            # non-critical singles are deferred into chunk-1 emission below.
            wqk_sb = singles.tile([128, NKT, 512], BF16)
            wqk_src = wqkT.rearrange("(kt p) j -> p kt j", p=128)
            for kt in range(NKT):
                nc.gpsimd.dma_start(wqk_sb[:, kt, :], wqk_src[:, kt, :])
            wv_sb = singles.tile([128, NKT, 256], BF16)
            wv_src = wvT.rearrange("(kt p) j -> p kt j", p=128)
            bqk_sb = singles.tile([128, 4], F32)
            bqksw_sb = singles.tile([128, 4], F32)
            bv_sb = singles.tile([128, 2], F32)
            cos_sb = singles.tile([128, S], BF16)
            sin_sb = singles.tile([128, S], BF16)
            ones_sb = singles.tile([128, 128], BF16)

            def emit_deferred_singles():
                for kt in range(0, NKT, 8):
                    nc.gpsimd.dma_start(wv_sb[:, kt:kt + 8, :],
                                        wv_src[:, kt:kt + 8, :])
                nc.gpsimd.dma_start(bqk_sb, bqk_d)
                nc.gpsimd.dma_start(bqksw_sb, bqksw_d)
                nc.gpsimd.dma_start(bv_sb, bv_d)
                nc.gpsimd.dma_start(cos_sb[:, 0:CHUNK], cos_d[:, 0:CHUNK])
                nc.gpsimd.dma_start(sin_sb[:, 0:CHUNK], sin_d[:, 0:CHUNK])
                nc.gpsimd.dma_start(cos_sb[:, CHUNK:], cos_d[:, CHUNK:])
                nc.gpsimd.dma_start(sin_sb[:, CHUNK:], sin_d[:, CHUNK:])
                nc.vector.memset(ones_sb, 1.0)

            # persistent per-core activations
            qkT_sb = singles.tile([128, 4, T], BF16)     # roped q/k, [hd, m, tok]
            v_sb = singles.tile([128, T // 128, 256], BF16)  # v natural

            cur = {}  # phase-dependent pool selection for attention quanta

            with tc.tile_pool(name="pt", bufs=10) as ptp, \
                 tc.tile_pool(name="rs", bufs=6) as rsp, \
                 tc.tile_pool(name="ao", bufs=3) as aop, \
                 tc.tile_pool(name="ps_o", bufs=4, space="PSUM") as ps_o:

                def attn_units(b, hl, qc):
                    """Return [(ready_after_chunk, kind, thunk), ...]."""
                    tok0 = b * S + qc * QCHUNK
                    st = {}
                    cbase = 4 * b  # first chunk index of this batch

                    def score_half(jj):
                        if jj == 0:
                            st["o"] = ps_o.tile([128, QCHUNK], F32, tag="o",
                                                name="o_ps")
                        s_ps = cur["sp"].tile([128, 1024], F32, name="s_ps")
                        for u in (0, 1):
                            j = 2 * jj + u
                            nc.tensor.matmul(
                                s_ps[:, ts(u, 512)],
                                lhsT=qkT_sb[:, 2 + hl,
                                            b * S + j * 128:b * S + (j + 1) * 128],
                                rhs=qkT_sb[:, hl, tok0:tok0 + QCHUNK],
                                start=True, stop=True)
                        p_sb = ptp.tile([128, 1024], BF16, name="p_sb")
                        nc.scalar.activation(p_sb, s_ps, Exp, scale=SCALE)
                        st.setdefault("p", {})[jj] = p_sb

                    def pv_half(jj):
                        p_sb = st["p"].pop(jj)
                        for u in (0, 1):
                            j = 2 * jj + u
                            nc.tensor.matmul(
                                st["o"],
                                lhsT=v_sb[:, b * NJ + j, ts(hl, 128)],
                                rhs=p_sb[:, ts(u, 512)],
                                start=(j == 0), stop=(j == NJ - 1))
                        if jj % 2 == 0:
                            st["pend"] = p_sb
                        elif "acc" not in st:
                            acc = rsp.tile([128, 1024], BF16, tag="acc")
                            nc.vector.tensor_add(acc, st["pend"], p_sb)
                            st["acc"] = acc
                        else:
                            tmp = rsp.tile([128, 1024], BF16, tag="tp")
                            nc.vector.tensor_add(tmp, st["pend"], p_sb)
                            nacc = rsp.tile([128, 1024], BF16, tag="acc")
                            nc.vector.tensor_add(nacc, st["acc"], tmp)
                            st["acc"] = nacc

                    def close():
                        tf = rsp.tile([128, QCHUNK], BF16, tag="tf")
                        nc.vector.tensor_add(
                            tf, st["acc"][:, 0:512], st["acc"][:, 512:1024])
                        # rowsum matmul borrows a scores-pool ring slot
                        r_ps = cur["sp"].tile([128, 1024], F32, name="s_ps")
                        nc.tensor.matmul(r_ps[:, 0:512], lhsT=ones_sb, rhs=tf,
                                         start=True, stop=True)
                        recip = aop.tile([128, QCHUNK], F32, tag="recip")
                        nc.vector.reciprocal_approx_fast(recip, r_ps[:, 0:512])
                        o1 = aop.tile([128, QCHUNK], F32, tag="o1")
                        nc.vector.tensor_mul(o1, st["o"], recip)
                        o2 = aop.tile([128, QCHUNK], F32, tag="o2")
                        nc.gpsimd.tensor_add(
                            o2, o1,
                            bv_sb[:, hl:hl + 1].broadcast_to([128, QCHUNK]))
                        nc.sync.dma_start(
                            out_d[ts(hl, 128), tok0:tok0 + QCHUNK], o2)

                    out = []
                    for jj in range(NJ // 2):
                        ready = cbase + max(qc, (2 * jj + 1) // 4)
                        out.append((ready, "unit",
                                    (lambda jj=jj: score_half(jj),
                                     lambda jj=jj: pv_half(jj))))
                    out.append((cbase + 3, "close", close))
                    return out

                # combos in open order: qc-major within batch, hl inner
                combos = [attn_units(b, hl, qc)
                          for b in range(B)
                          for qc in range(S // QCHUNK)
                          for hl in range(2)]
                NCOMB = len(combos)

                # Greedy window scheduler: combo k may open only when combo
                # k-4 has closed (PSUM o-ring depth 4); quanta within a combo
                # stay sequential; <=18 quanta per projection window.
                ptr = [0] * NCOMB
                closed_upto = 0  # combos fully scheduled
                win_sched = {w: [] for w in range(1, 8)}
                for w in range(1, 8):
                    cap = 18
                    progress = True
                    while cap > 0 and progress:
                        progress = False
                        for ci in range(NCOMB):
                            if cap == 0:
                                break
                            if ptr[ci] >= len(combos[ci]):
                                continue
                            if ptr[ci] == 0 and ci >= 4 and \
                                    ptr[ci - 4] < len(combos[ci - 4]):
                                continue  # o-ring slot still held
                            ready, kind, pl = combos[ci][ptr[ci]]
                            if ready > w - 1:
                                continue
                            win_sched[w].append((kind, pl))
                            ptr[ci] += 1
                            cap -= 1
                            progress = True
                drain_q = []
                for ci in range(NCOMB):
                    for item in combos[ci][ptr[ci]:]:
                        drain_q.append(item)

                # one-quantum software pipeline: scores half runs one slot
                # ahead of its pv half so the exp latency is always covered
                pipe = {"b": None}

                def emit_q(item):
                    kind, pl = item
                    if kind == "unit":
                        a_th, b_th = pl
                        a_th()
                        if pipe["b"] is not None:
                            pipe["b"]()
                        pipe["b"] = b_th
                    else:
                        if pipe["b"] is not None:
                            pipe["b"]()
                            pipe["b"] = None
                        pl()

                def pop_ready(tci, kmax):
                    lst = win_sched.get(tci)
                    if not lst:
                        return
                    for item in lst[:kmax]:
                        emit_q(item)
                    del lst[:kmax]

                def emit_qkv_chunk(tci, xcp, wkp, ps_qk, ps_v):
                    pos0 = (tci % (S // CHUNK)) * CHUNK
                    xc = xcp.tile([128, NKT, CHUNK], BF16, name="xc")
                    xc_src = xT[:, ts(tci, CHUNK)].rearrange(
                        "(kt p) t -> p kt t", p=128)
                    if tci == 0:
                        # fine-grained for fast rampup, two trigger engines
                        for kt in range(NKT):
                            eng = nc.sync if kt % 2 == 0 else nc.scalar
                            eng.dma_start(xc[:, kt, :], xc_src[:, kt, :])
                        emit_deferred_singles()
                    else:
                        for kt in range(0, NKT, 4):
                            nc.sync.dma_start(xc[:, kt:kt + 4, :],
                                              xc_src[:, kt:kt + 4, :])
                    qk_raw = wkp.tile([128, 4, CHUNK], BF16, tag="raw")
                    qk_sw = wkp.tile([128, 4, CHUNK], BF16, tag="sw")
                    for m in range(4):
                        # qk chain for m
                        pq = ps_qk.tile([128, CHUNK], F32, name="pq")
                        for kt in range(NKT):
                            nc.tensor.matmul(
                                pq, lhsT=wqk_sb[:, kt, ts(m, 128)],
                                rhs=xc[:, kt, :],
                                start=(kt == 0), stop=(kt == NKT - 1))
                            if kt == 7:
                                pop_ready(tci, 1)
                        nc.vector.tensor_copy(qk_raw[:, m, :], pq)
                        pop_ready(tci, 1)
                        # v chain for token tile m
                        pv = ps_v.tile([128, 256], F32, name="pv")
                        for kt in range(NKT):
                            nc.tensor.matmul(
                                pv, lhsT=xc[:, kt, ts(m, 128)],
                                rhs=wv_sb[:, kt, :],
                                start=(kt == 0), stop=(kt == NKT - 1))
                            if kt == 7:
                                pop_ready(tci, 1)
                        nc.vector.tensor_copy(
                            v_sb[:, tci * (CHUNK // 128) + m, :], pv)
                        pop_ready(tci, 1)
                    # 64-partition block swap (rotate-half partner), all 4 m
                    nc.gpsimd.dma_start(qk_sw[0:64, :, :], qk_raw[64:128, :, :])
                    nc.gpsimd.dma_start(qk_sw[64:128, :, :], qk_raw[0:64, :, :])
                    for m in range(4):
                        # rope with fused bias:
                        #   y = (x+b)*cos + (swap(x)+swap(b))*sin'
                        t1 = wkp.tile([128, CHUNK], BF16, tag="t1")
                        t2 = wkp.tile([128, CHUNK], BF16, tag="t2")
                        nc.vector.scalar_tensor_tensor(
                            t1, qk_raw[:, m, :], bqk_sb[:, m:m + 1],
                            cos_sb[:, pos0:pos0 + CHUNK],
                            op0=mybir.AluOpType.add, op1=mybir.AluOpType.mult)
                        nc.vector.scalar_tensor_tensor(
                            t2, qk_sw[:, m, :], bqksw_sb[:, m:m + 1],
                            sin_sb[:, pos0:pos0 + CHUNK],
                            op0=mybir.AluOpType.add, op1=mybir.AluOpType.mult)
                        nc.vector.tensor_add(
                            qkT_sb[:, m, ts(tci, CHUNK)], t1, t2)
                    pop_ready(tci, 2)

                # ---- projection phase with interleaved attention ----
                with tc.tile_pool(name="xc", bufs=2) as xcp, \
                     tc.tile_pool(name="work", bufs=2) as wkp, \
                     tc.tile_pool(name="ps_qk", bufs=1, space="PSUM") as ps_qk, \
                     tc.tile_pool(name="ps_v", bufs=1, space="PSUM") as ps_v, \
                     tc.tile_pool(name="ps_sA", bufs=1, space="PSUM") as ps_sA:
                    cur["sp"] = ps_sA
                    for tci in range(8):
                        emit_qkv_chunk(tci, xcp, wkp, ps_qk, ps_v)
                        for item in win_sched.get(tci, []):
                            emit_q(item)  # flush stragglers at chunk boundary
                        win_sched[tci] = []

                # ---- drain phase: leftover quanta, delayed closes ----
                with tc.tile_pool(name="ps_sC", bufs=2, space="PSUM") as ps_sC:
                    cur["sp"] = ps_sC
                    pending = deque()  # [close_thunk, units_to_wait]
                    for ready, kind, pl in drain_q:
                        if kind == "close":
                            pending.append([pl, 2])
                            continue
                        emit_q(("unit", pl))
                        for item in pending:
                            item[1] -= 1
                        while pending and pending[0][1] <= 0:
                            emit_q(("close", pending.popleft()[0]))
                    while pending:
                        emit_q(("close", pending.popleft()[0]))
                    if pipe["b"] is not None:
                        pipe["b"]()
                        pipe["b"] = None

    nc.compile()
    _prog_cache["nc"] = nc
    return nc


_PERM = np.concatenate([np.arange(0, 128, 2), np.arange(1, 128, 2)])


def _prep_inputs(sequence, frequencies, Wq, bq, Wk, bk, Wv, bv):
    bf = ml_dtypes.bfloat16
    x = np.ascontiguousarray(sequence.reshape(T, D))
    xT = np.ascontiguousarray(x.T).astype(bf)

    i_idx = np.arange(128) % 64
    ang = np.asarray(frequencies, np.float32)
    cos_g = np.ascontiguousarray(np.cos(ang[:, i_idx]).T)
    sin_g = np.ascontiguousarray(np.sin(ang[:, i_idx]).T)
    sin_g[:64] *= -1.0
    cos_g = cos_g.astype(bf)
    sin_g = sin_g.astype(bf)

    in_maps = []
    for c in range(NCORES):
        h0, h1 = 2 * c, 2 * c + 1
        WQK = np.concatenate(
            [Wq[h * 128:(h + 1) * 128][_PERM] for h in (h0, h1)]
            + [Wk[h * 128:(h + 1) * 128][_PERM] for h in (h0, h1)], 0)
        bqk = np.concatenate(
            [bq[h * 128:(h + 1) * 128][_PERM] for h in (h0, h1)]
            + [bk[h * 128:(h + 1) * 128][_PERM] for h in (h0, h1)])
        WV = np.concatenate([Wv[h * 128:(h + 1) * 128] for h in (h0, h1)], 0)
        bvc = np.concatenate([bv[h * 128:(h + 1) * 128] for h in (h0, h1)])
        in_maps.append({
            "xT": xT,
            "wqkT": np.ascontiguousarray(WQK.T).astype(bf),
            "wvT": np.ascontiguousarray(WV.T).astype(bf),
            "bqk": np.ascontiguousarray(bqk.reshape(4, 128).T).astype(np.float32),
            "bqksw": np.ascontiguousarray(
                np.roll(bqk.reshape(4, 128), 64, axis=1).T).astype(np.float32),
            "bv": np.ascontiguousarray(bvc.reshape(2, 128).T).astype(np.float32),
            "cosg": cos_g,
            "sing": sin_g,
        })
    return in_maps


def kernel(sequence, frequencies, mask, Wq, bq, Wk, bk, Wv, bv):
    sequence = np.asarray(sequence, np.float32)
    frequencies = np.asarray(frequencies, np.float32)
    Wq, bq = np.asarray(Wq, np.float32), np.asarray(bq, np.float32)
    Wk, bk = np.asarray(Wk, np.float32), np.asarray(bk, np.float32)
    Wv, bv = np.asarray(Wv, np.float32), np.asarray(bv, np.float32)
    nc = _build_program()
    in_maps = _prep_inputs(sequence, frequencies, Wq, bq, Wk, bk, Wv, bv)
    trace = bool(int(os.environ.get("BENCH_TRACE", "0")))
    res = run_bass_kernel_spmd(nc, in_maps, list(range(NCORES)), trace=trace)
    _last_results["exec_time_ns"] = res.exec_time_ns
    _last_results["results"] = res

    out = np.empty((B, S, D), np.float32)
    for c in range(NCORES):
        oc = res.results[c]["out"]           # [256, T]
        for hl in range(2):
            h = 2 * c + hl
            for b in range(B):
                out[b, :, h * 128:(h + 1) * 128] = \
                    oc[hl * 128:(hl + 1) * 128, b * S:(b + 1) * S].T
    return out
